# revision 4
# baseline (speedup 1.0000x reference)
"""AvULoss (Accuracy-vs-Uncertainty loss) TRN2 Bass kernel, v2.

Full inputs:  logits [2097152, 32] f32, labels [2097152] i64, unc_th [] f32.
Output: avu_loss [1] f32.

Data-parallel over rows across 8 cores. Host-side sharding additionally
SORTS rows by label into fixed per-label bands (identical band layout on
every core, so one SPMD program serves all 8): band c holds only rows whose
label is c, at u_c positions per partition. Row order does not change the
four reduction sums, so this is purely a layout choice. With it, per band,
    accurate = (x[:, f, c_band] == rowmax)        (exact f32 compare)
needs no argmax extraction: rowmax is a plain segmented reduce_max and
x[:, f, c_band] is a strided slice at a compile-time class index.

Rows that exceed their band's capacity (multinomial remainders, ~4k of 2M)
go to a small overflow region processed with the classic pack trick
((x|31)^(31-c) -> reduce_max gives max AND argmax). Band deficits and
overflow padding are zero-logit dummy rows with label 31; each contributes
exactly num += 0 and den += tanh(ln 32)/32 (uniform softmax, accurate=1,
uncertain), which the host subtracts analytically.

Per row: s = sum_c e^x (PE, PSUM-accumulating identity matmuls on bf16 e),
d = sum_c x e^x (PE, on bf16 x*e), mx = max_c x (DVE segmented reduce).
Tail avoids Ln/Tanh tables (ACT runs Exp only):
    unc <= th  <=>  v <= e^{2 th},  v := s^2 e^{-2 d/s} = e^{2 unc}
    tanh(unc)  =   1 - 2/(v+1)
Engine split per tile, chosen from HW microbenchmarks of every op class:
ACT: exp, band copies, PSUM evac, affine ops; DVE: reduce_max, reciprocals,
predicated copies, compares, small muls + a slice of x*e; Pool: most of
x*e and the tail muls; PE: s/d chains, two tiles interleaved (4 chains in
flight) to hide PSUM read-modify-write latency; SP: all DMAs, each tile's
load split into 1MB pieces (measured ~2x DMA bandwidth vs one 4MB copy).
"""

import math

import numpy as np

import concourse.bass as bass
import concourse.bacc as bacc
import concourse.tile as tile
from concourse import mybir
from concourse.bass_utils import run_bass_kernel_spmd

N_FULL = 2097152
C = 32
N_CORES = 8
P = 128
R = 256          # row-positions per partition per tile
NT = 8           # tiles (band region F = NT*R = 2048 positions)
F = NT * R
EPS = 1e-10
BETA = 1.0
XD = 6           # classes of x*e computed on DVE; the rest on Pool

F32 = mybir.dt.float32
U32 = mybir.dt.uint32
BF16 = mybir.dt.bfloat16
AX = mybir.AxisListType.X
ALU = mybir.AluOpType
ACT_F = mybir.ActivationFunctionType

# den contribution of one dummy row (uniform softmax over 32 classes):
# conf=1/32, accurate=1 (0==0 tie), unc=ln32>th -> f1=conf, f2=tanh(ln32).
DUMMY_DEN = math.tanh(math.log(32.0)) / 32.0


def _ts_u32imm(eng, out, in0, imm, op0, in1=None, op1=ALU.bypass):
    """TensorScalarPtr with a uint32-typed immediate (bitvec ops on u32)."""
    ins = [eng.lower_ap(in0), mybir.ImmediateValue(dtype=U32, value=imm)]
    if in1 is not None:
        ins.append(eng.lower_ap(in1))
    return eng.add_instruction(
        mybir.InstTensorScalarPtr(
            name=eng.bass.get_next_instruction_name(),
            is_scalar_tensor_tensor=in1 is not None,
            op0=op0,
            op1=op1,
            ins=ins,
            outs=[eng.lower_ap(out)],
        )
    )


def plan_layout(labels: np.ndarray):
    """Band widths u_c (positions per partition), sum u == F."""
    counts = np.bincount(labels, minlength=C).astype(np.int64)
    per_pos = P * N_CORES  # rows per global position
    u = counts // per_pos
    rem = F - int(u.sum())
    frac = counts - u * per_pos
    order = np.argsort(-frac)
    u[order[:rem]] += 1
    assert int(u.sum()) == F and (u >= 0).all()
    starts = np.zeros(C + 1, np.int64)
    starts[1:] = np.cumsum(u)
    bands = tuple(
        (int(starts[c]), int(u[c]), c) for c in range(C) if u[c] > 0
    )
    return u, starts, bands


def shard_inputs(logits: np.ndarray, labels: np.ndarray, unc_th):
    """Sort rows into label bands + overflow; build per-core inputs."""
    labels = np.asarray(labels).astype(np.int64)
    u, starts, bands = plan_layout(labels)
    per_pos = P * N_CORES

    order = np.argsort(labels, kind="stable")
    lab_sorted = labels[order]
    first = np.searchsorted(lab_sorted, np.arange(C))
    last = np.searchsorted(lab_sorted, np.arange(C), side="right")

    # grid[k, p, f] = source row id (-1 => dummy) for band region
    grid = np.full((N_CORES, P, F), -1, np.int64)
    ov_list = []
    n_band_dummy = 0
    for c in range(C):
        ids = order[first[c] : last[c]]
        cap = int(u[c]) * per_pos
        take = min(len(ids), cap)
        a = np.full(cap, -1, np.int64)
        a[:take] = ids[:take]
        n_band_dummy += cap - take
        grid[:, :, starts[c] : starts[c + 1]] = a.reshape(N_CORES, P, int(u[c]))
        if len(ids) > cap:
            ov_list.append(ids[cap:])

    ov = (
        np.concatenate(ov_list)
        if ov_list
        else np.zeros(0, np.int64)
    )
    OV = max(1, -(-len(ov) // per_pos))  # ceil
    ov_pad = np.full(OV * per_pos, -1, np.int64)
    ov_pad[: len(ov)] = ov
    n_ov_dummy = OV * per_pos - len(ov)
    ov_grid = ov_pad.reshape(N_CORES, P, OV)

    n_dummy = n_band_dummy + n_ov_dummy

    logits_pad = np.vstack([logits, np.zeros((1, C), np.float32)])
    lab_pad = np.concatenate([labels, [31]]).astype(np.uint32)  # dummy label 31

    th = np.array([[np.float32(unc_th)]], dtype=np.float32)
    in_maps = []
    for k in range(N_CORES):
        idx = np.concatenate([grid[k], ov_grid[k]], axis=1).reshape(-1)  # [P*(F+OV)]
        xs = np.ascontiguousarray(logits_pad[idx])  # [(F+OV)*P, C] p-major
        ol = np.ascontiguousarray(lab_pad[ov_grid[k].reshape(-1)])  # [P*OV]
        in_maps.append({"logits": xs, "ovlab": ol, "th": th})
    return in_maps, OV, bands, n_dummy


def build_nc(bands, OV: int, reps: int = 1):
    """Per-core Bass program. bands: ((start, width, class), ...) in
    position space [0, F); OV: overflow positions per partition."""
    FT = F + OV
    nc = bacc.Bacc("TRN2", target_bir_lowering=False, debug=False)
    x_d = nc.dram_tensor("logits", [FT * P, C], F32, kind="ExternalInput").ap()
    ol_d = nc.dram_tensor("ovlab", [P * OV], U32, kind="ExternalInput").ap()
    th_d = nc.dram_tensor("th", [1, 1], F32, kind="ExternalInput").ap()
    out_d = nc.dram_tensor("partials", [1, 2], F32, kind="ExternalOutput").ap()

    xt = x_d.rearrange("(p f) c -> p f c", p=P)  # [P, FT, C]
    olt = ol_d.rearrange("(p f) -> p f", p=P)    # [P, OV]

    NG = NT // 2  # tail groups of 2 tiles -> [P, 512] tail ops

    with tile.TileContext(nc) as tc:
        with (
            tc.tile_pool(name="xin", bufs=2) as xin,
            tc.tile_pool(name="work", bufs=2) as work,
            tc.tile_pool(name="slabs", bufs=1) as slabs,
            tc.tile_pool(name="gw", bufs=2) as gw,
            tc.tile_pool(name="singles", bufs=1) as singles,
            tc.tile_pool(name="psum", bufs=1, space="PSUM") as psum_pool,
            tc.tile_pool(name="psum1", bufs=1, space="PSUM") as psum1,
        ):
            # ---- constants ----
            th_sb = singles.tile([P, 1], F32)
            th_bcast = bass.AP(
                tensor=th_d.tensor, offset=th_d.offset, ap=[[0, P], [1, 1]]
            )
            nc.sync.dma_start(th_sb[:], th_bcast)
            e2_sb = singles.tile([P, 1], F32)
            nc.scalar.activation(e2_sb[:], th_sb[:], ACT_F.Exp, scale=2.0)
            ones_sb = singles.tile([P, 1], F32)
            nc.vector.memset(ones_sb[:], 1.0)
            identd = singles.tile([P, P], mybir.dt.int32)
            nc.gpsimd.iota(identd[:], pattern=[[1, P]], base=0, channel_multiplier=-1)
            ident = singles.tile([P, P], BF16)
            nc.vector.tensor_scalar(ident[:], identd[:], 0, None, op0=ALU.is_equal)
            # overflow constants: iotax[c] = 31 - c; labels
            iotax_g = singles.tile([P, C], U32)
            nc.gpsimd.iota(iotax_g[:], pattern=[[-1, C]], base=31, channel_multiplier=0)
            iotax = singles.tile([P, C], U32)
            nc.vector.tensor_copy(iotax[:], iotax_g[:])
            ovlab_sb = singles.tile([P, OV], U32)
            nc.sync.dma_start(ovlab_sb[:], olt)

            # ---- slabs ----
            mx_sl = slabs.tile([P, F], F32)
            xl_sl = slabs.tile([P, F], F32)
            nd = slabs.tile([P, 2, NG + 1], F32)  # [num|den] per group + overflow

            for _ in range(reps):
                ps_s = ps_d = None
                for k in range(NT):
                    sl = slice(k * R, (k + 1) * R)
                    g, kk = divmod(k, 2)
                    x = xin.tile([P, R, C], F32)
                    # tile load split into 1MB pieces (2x DMA bandwidth)
                    q = R // 4
                    for j in range(4):
                        nc.sync.dma_start(
                            x[:, j * q : (j + 1) * q, :],
                            xt[:, k * R + j * q : k * R + (j + 1) * q, :],
                        )
                    # e = exp(x) -> bf16 ; ex = x*e -> bf16
                    es = work.tile([P, R, C], BF16, tag="es")
                    nc.scalar.activation(es[:], x[:], ACT_F.Exp)
                    exs = work.tile([P, R, C], BF16, tag="exs")
                    nc.vector.tensor_mul(
                        exs[:, :, 0:XD], x[:, :, 0:XD], es[:, :, 0:XD]
                    )
                    nc.gpsimd.tensor_mul(
                        exs[:, :, XD:C], x[:, :, XD:C], es[:, :, XD:C]
                    )
                    # row max (exact, f32)
                    nc.vector.reduce_max(mx_sl[:, sl], x[:], axis=AX)
                    # xl: per band piece, copy the band-class column
                    for (b0, w, cb) in bands:
                        f0, f1 = max(b0, k * R), min(b0 + w, (k + 1) * R)
                        if f0 < f1:
                            nc.scalar.copy(
                                xl_sl[:, f0:f1],
                                x[:, f0 - k * R : f1 - k * R, cb],
                            )
                    # s/d chains; pair tiles -> 4 chains in flight, each in
                    # its own PSUM bank (interleaving two accumulation regions
                    # inside one bank is slow and unreliable)
                    if kk == 0:
                        ps_sA = psum_pool.tile([P, 512], F32, tag="ps_sA")
                        ps_dA = psum_pool.tile([P, 512], F32, tag="ps_dA")
                        ps_sB = psum_pool.tile([P, 512], F32, tag="ps_sB")
                        ps_dB = psum_pool.tile([P, 512], F32, tag="ps_dB")
                        pend = (x, es, exs, k)
                        continue
                    px, pes, pexs, pk = pend
                    for c in range(C):
                        st, sp = c == 0, c == C - 1
                        nc.tensor.matmul(
                            ps_sA[:, 0:R], ident[:], pes[:, :, c], start=st, stop=sp
                        )
                        nc.tensor.matmul(
                            ps_dA[:, 0:R], ident[:], pexs[:, :, c], start=st, stop=sp
                        )
                        nc.tensor.matmul(
                            ps_sB[:, 0:R], ident[:], es[:, :, c], start=st, stop=sp
                        )
                        nc.tensor.matmul(
                            ps_dB[:, 0:R], ident[:], exs[:, :, c], start=st, stop=sp
                        )

                    # ---- tail for this pair on [P, 2R] ----
                    W = 2 * R
                    gsl = slice(g * W, (g + 1) * W)
                    sc = gw.tile([P, W], F32, tag="sc")
                    nc.scalar.copy(sc[:, 0:R], ps_sA[:, 0:R])
                    nc.scalar.copy(sc[:, R:W], ps_sB[:, 0:R])
                    dc = gw.tile([P, W], F32, tag="dc")
                    nc.scalar.copy(dc[:, 0:R], ps_dA[:, 0:R])
                    nc.scalar.copy(dc[:, R:W], ps_dB[:, 0:R])
                    rs = gw.tile([P, W], F32, tag="rs")
                    nc.vector.reciprocal_approx_fast(rs[:], sc[:])
                    # dc <- y = d/s; dc <- e^{-2y}; dc <- v = s^2 e^{-2y}
                    nc.gpsimd.tensor_mul(dc[:], dc[:], rs[:])
                    nc.scalar.activation(dc[:], dc[:], ACT_F.Exp, scale=-2.0)
                    nc.gpsimd.tensor_mul(dc[:], dc[:], sc[:])
                    nc.gpsimd.tensor_mul(dc[:], dc[:], sc[:])
                    v = dc
                    cc = gw.tile([P, W], F32, tag="cc")
                    nc.vector.tensor_scalar(cc[:], v[:], e2_sb[:], None, op0=ALU.is_le)
                    # v <- v+1; r2 = 1/(v+1); t = 1-2r2 (=tanh(unc)); r2 <- 2r2 (=1-t)
                    nc.scalar.activation(v[:], v[:], ACT_F.Identity, bias=1.0)
                    r2 = gw.tile([P, W], F32, tag="r2")
                    nc.vector.reciprocal_approx_fast(r2[:], v[:])
                    t = gw.tile([P, W], F32, tag="t")
                    nc.scalar.activation(t[:], r2[:], ACT_F.Identity, scale=-2.0, bias=1.0)
                    nc.scalar.mul(r2[:], r2[:], 2.0)
                    a = gw.tile([P, W], F32, tag="a")
                    nc.vector.tensor_tensor(
                        a[:], xl_sl[:, gsl], mx_sl[:, gsl], op=ALU.is_equal
                    )
                    cf = gw.tile([P, W], F32, tag="cf")
                    nc.scalar.activation(cf[:], mx_sl[:, gsl], ACT_F.Exp)
                    nc.gpsimd.tensor_mul(cf[:], cf[:], rs[:])
                    conf = cf
                    f1 = gw.tile([P, W], F32, tag="f1")
                    nc.scalar.activation(f1[:], conf[:], ACT_F.Identity, scale=-1.0, bias=1.0)
                    nc.vector.copy_predicated(f1[:], a[:].bitcast(U32), conf[:])
                    nc.vector.copy_predicated(t[:], cc[:].bitcast(U32), r2[:])
                    nc.vector.tensor_mul(f1[:], f1[:], t[:])
                    den = f1
                    nc.vector.tensor_tensor(a[:], a[:], cc[:], op=ALU.is_equal)
                    nc.vector.tensor_mul(a[:], den[:], a[:])
                    num = a
                    nc.vector.reduce_sum(nd[:, 0, g : g + 1], num[:], axis=AX)
                    nc.vector.reduce_sum(nd[:, 1, g : g + 1], den[:], axis=AX)

                # ---- overflow region: pack-trick path on [P, OV, C] ----
                xo = xin.tile([P, OV, C], F32, tag="xo")
                nc.sync.dma_start(xo[:], xt[:, F : F + OV, :])
                eo = work.tile([P, OV, C], BF16, tag="eo")
                nc.scalar.activation(eo[:], xo[:], ACT_F.Exp)
                exo = work.tile([P, OV, C], BF16, tag="exo")
                nc.vector.tensor_mul(exo[:], xo[:], eo[:])
                so = work.tile([P, OV], F32, tag="so")
                nc.vector.reduce_sum(so[:], eo[:], axis=AX)
                do = work.tile([P, OV], F32, tag="do")
                nc.vector.reduce_sum(do[:], exo[:], axis=AX)
                iotax_b = iotax[:].unsqueeze(1).broadcast_to([P, OV, C])
                _ts_u32imm(
                    nc.vector,
                    xo[:].bitcast(U32),
                    xo[:].bitcast(U32),
                    31,
                    ALU.bitwise_or,
                    in1=iotax_b,
                    op1=ALU.bitwise_xor,
                )
                mo = work.tile([P, OV], F32, tag="mo")
                nc.vector.reduce_max(mo[:], xo[:], axis=AX)
                # a = ((mx & 31) == label)
                ao = work.tile([P, OV], F32, tag="ao")
                _ts_u32imm(
                    nc.vector, ao[:].bitcast(U32), mo[:].bitcast(U32), 31, ALU.bitwise_and
                )
                nc.vector.tensor_tensor(
                    ao[:], ao[:].bitcast(U32), ovlab_sb[:], op=ALU.is_equal
                )
                rso = work.tile([P, OV], F32, tag="rso")
                nc.vector.reciprocal_approx_fast(rso[:], so[:])
                yo = work.tile([P, OV], F32, tag="yo")
                nc.vector.tensor_mul(yo[:], do[:], rso[:])
                wo = work.tile([P, OV], F32, tag="wo")
                nc.scalar.activation(wo[:], yo[:], ACT_F.Exp, scale=-2.0)
                nc.vector.tensor_mul(wo[:], wo[:], so[:])
                nc.vector.tensor_mul(wo[:], wo[:], so[:])  # wo = v
                vpo = work.tile([P, OV], F32, tag="vpo")
                nc.scalar.activation(vpo[:], wo[:], ACT_F.Identity, bias=1.0)
                r2o = work.tile([P, OV], F32, tag="r2o")
                nc.vector.reciprocal_approx_fast(r2o[:], vpo[:])
                to = work.tile([P, OV], F32, tag="to")
                nc.scalar.activation(to[:], r2o[:], ACT_F.Identity, scale=-2.0, bias=1.0)
                tmo = work.tile([P, OV], F32, tag="tmo")
                nc.scalar.mul(tmo[:], r2o[:], 2.0)
                cco = work.tile([P, OV], F32, tag="cco")
                nc.vector.tensor_scalar(cco[:], wo[:], e2_sb[:], None, op0=ALU.is_le)
                cfo = work.tile([P, OV], F32, tag="cfo")
                nc.scalar.activation(cfo[:], mo[:], ACT_F.Exp)
                confo = work.tile([P, OV], F32, tag="confo")
                nc.vector.tensor_mul(confo[:], cfo[:], rso[:])
                f1o = work.tile([P, OV], F32, tag="f1o")
                nc.scalar.activation(f1o[:], confo[:], ACT_F.Identity, scale=-1.0, bias=1.0)
                nc.vector.copy_predicated(f1o[:], ao[:].bitcast(U32), confo[:])
                nc.vector.copy_predicated(to[:], cco[:].bitcast(U32), tmo[:])
                nc.vector.tensor_mul(f1o[:], f1o[:], to[:])
                nc.vector.tensor_tensor(ao[:], ao[:], cco[:], op=ALU.is_equal)
                nc.vector.tensor_mul(ao[:], f1o[:], ao[:])
                nc.vector.reduce_sum(nd[:, 0, NG : NG + 1], ao[:], axis=AX)
                nc.vector.reduce_sum(nd[:, 1, NG : NG + 1], f1o[:], axis=AX)

            # ---- final: sum groups, cross-partition reduce ----
            ndr = singles.tile([P, 2], F32)
            nc.vector.reduce_sum(ndr[:], nd[:], axis=AX)
            ps = psum1.tile([1, 2], F32)
            nc.tensor.matmul(ps[:], ones_sb[:], ndr[:], start=True, stop=True)
            out_sb = singles.tile([1, 2], F32)
            nc.scalar.copy(out_sb[:], ps[:])
            nc.sync.dma_start(out_d, out_sb[:])

    nc.compile()
    return nc


_NC_CACHE: dict = {}


def kernel(logits, labels, unc_th, _trace: bool = False):
    logits = np.asarray(logits, dtype=np.float32)
    in_maps, OV, bands, n_dummy = shard_inputs(logits, np.asarray(labels), unc_th)

    key = (bands, OV)
    if key not in _NC_CACHE:
        _NC_CACHE[key] = build_nc(bands, OV)
    nc = _NC_CACHE[key]

    res = run_bass_kernel_spmd(
        nc, in_maps, core_ids=list(range(N_CORES)), trace=_trace
    )
    num = np.float64(0.0)
    den = np.float64(0.0)
    for r in res.results:
        p = r["partials"].reshape(-1)
        num += np.float64(p[0])
        den += np.float64(p[1])
    den -= n_dummy * DUMMY_DEN
    avu = num / (den + EPS)
    loss = -BETA * np.log(avu + EPS)
    out = np.array([loss], dtype=np.float32)
    if _trace:
        return out, res
    return out
